# revision 19
# baseline (speedup 1.0000x reference)
"""Trainium2 Bass kernel for nn_Attention_Embedding (spatial NxN attention +
channel CxC attention + conv3d(1,1,4) embedding head).

Sharding: 8 cores = 4 samples x 2 halves (split on H). Each core holds its
sample's full q (softmax rows are complete) and produces its own slice of the
final output; no cross-core communication.

v8.5 (~94.8-99.2us measured vs 96.9-98.6 for v7; run-to-run HAM-phase
noise is ~±2us). The wall clock is bound by PSUM-egress on ACT+DVE (the
64 [128,1024] exp tiles at ~1.0/1.2us each -> ~37us two-engine floor),
so everything else is stripped off those engines and out of the
head/tail/boundary:
  - host-baked qTP row-group replicas and qc2 [beta*data|ones]x2 (beta
    folded into the AV weights kills the recB normalize mul); inputs on
    3 parallel DMA queues with qTP split so S(0) starts early; the
    pre-stream gram burst fills the qTP wait and warms the HAM; ACT
    exp-table load fires at t~0 on a memset tile.
  - gram rhs includes the ones column (N=33) so csum falls out of the
    quadrant sums -- no 4.4us DVE reduce over qTP.
  - small1 (G -> attn2 -> wpa -> W) staged across 5 hooks so its serial
    DVE<->PE chain never head-blocks the PE queue.
  - convs use the 4-accumulating-K=32 kshift form reading caF/paF rows
    0:32 (no replica DMAs); each 512-chunk goes to an alternating PE
    col-group so consecutive chunks overlap; hooks emit one chunk at a
    time (a paired emission was a 5us PE wall at the half boundary).
    caF keeps a 4-col zero gap between halves (no WAR dep on norm B).
  - pa branch: W = gamma*wpa + [I;0] folded once, paF = ACT copies.
  - normalize: 1/denom via a one-op DVE exponent-flip bit trick
    (bitcast int32 add/mult, ~±5.5% -> ~1.4e-3 of output absmax since
    beta*ca is small); residual adds on GPSIMD (half A) / DVE (tail).
  - the last batch's AV matmuls carry over into the next half's stream
    (no half-boundary drain); tail runs norm-B p1 first so the cu=3
    ch-conv overlaps the p0 chain, cu3 borrowing the avA psum slot.
  - out is written [C, NL] (no on-chip transposes); host transposes.
"""

import os
import sys

for _p in ("/opt/trn_rl_repo", "/root/.axon_site/_ro/trn_rl_repo"):
    if os.path.isdir(_p) and _p not in sys.path:
        sys.path.insert(0, _p)
        break

import ml_dtypes
import numpy as np

import concourse.bacc as bacc
import concourse.bass as bass
import concourse.mybir as mybir
import concourse.tile as tile
from concourse import bass_utils

B, H, W, D, C = 4, 16, 16, 16, 32
N = H * W * D            # 4096
NL = N // 2              # 2048 rows per core
DO = D - 3               # 13 conv output positions
NCORES = 8
NJC = N // 128           # 32 j-chunks

f32 = mybir.dt.float32
f32r = mybir.dt.float32r
bf16 = mybir.dt.bfloat16
i16 = mybir.dt.int16
FT = mybir.ActivationFunctionType
ALU = mybir.AluOpType
PSUM = bass.MemorySpace.PSUM

# Schraudolph bf16 exp on DVE: i16(round(x*A16 + B16)) bitcast bf16 ~ exp(x).
LN2 = 0.6931471805599453
A16 = 128.0 / LN2
B16 = 127.0 * 128.0 - 4.46   # magic-c correction balances error to ~±3.5%

N_DVE_PER_HALF = 15          # of 32 exp tiles per half on DVE (rest ACT)

# packed-constant layouts
PK_WQ, PK_WK, PK_SMALL, PK_ID33 = 0, 32, 64, 68   # f32 pack
PKF = 100
PR_WVT, PR_ID32 = 0, 33                           # f32r pack
PRF = 65
PB_WCH4, PB_WPOS4 = 0, 128                        # bf16 flat k-major conv w
PBF = 256

GAP = 4                      # zero columns between caF halves


def _dve_pattern(n_tiles, n_dve):
    out, acc = [], 0
    for _ in range(n_tiles):
        acc += n_dve
        if acc >= n_tiles:
            acc -= n_tiles
            out.append(True)
        else:
            out.append(False)
    return out


def _emit(tc, nc, t, out_d):
    with (
        tc.tile_pool(name="const", bufs=1) as cp,
        tc.tile_pool(name="work", bufs=1) as wp,
    ):
        # ---- SBUF tiles ----
        qTP_r = cp.tile([128, N], bf16)         # q^T replicated x4 (host)
        qTloc_r = cp.tile([C + 1, NL], f32r)    # local q_aug^T (f32 bits)
        qc2_b = cp.tile([128, NJC, 128], bf16)  # [b*data|ones]x2 AV weights
        qc_f = cp.tile([128, NJC, C + 1], f32)  # gram operand
        pk = cp.tile([128, PKF], f32)
        pkr = cp.tile([C, PRF], f32r)
        pkb = cp.tile([C, PBF], bf16)

        qTloc_f = qTloc_r[0:C, 0:NL].bitcast(f32)
        wq_f = pk[0:C + 1, PK_WQ:PK_WQ + C]
        wk_f = pk[0:C + 1, PK_WK:PK_WK + C]
        bch_v = pk[0:C, PK_SMALL:PK_SMALL + 1]
        bpos_v = pk[0:C, PK_SMALL + 1:PK_SMALL + 2]
        gamma_v = pk[0:C + 1, PK_SMALL + 2:PK_SMALL + 3]
        id33_f = pk[0:C + 1, PK_ID33:PK_ID33 + C]
        wvT_r = pkr[0:C, PR_WVT:PR_WVT + C + 1]
        id32_r = pkr[0:C, PR_ID32:PR_ID32 + C]
        wch4_flat = pkb[0:C, PB_WCH4:PB_WCH4 + 4 * C]
        wpos4_flat = pkb[0:C, PB_WPOS4:PB_WPOS4 + 4 * C]

        # trigger the ACT exp table load immediately (~1.3us): no input dep
        warm = wp.tile([1, 1], f32)
        nc.vector.memset(warm[:], 0.0)
        nc.scalar.activation(warm[:], warm[:], FT.Exp)

        # ---- input DMAs: sync+gpsimd queues only (keep ACT free).
        # qTP split so S(0..7) can start after the first 1024 cols land;
        # qcf feeds the pre-stream gram burst (fills the qTP wait).
        nc.sync.dma_start(qTP_r[:, 0:2048], t["qTP"][:, 0:2048])
        nc.sync.dma_start(qTP_r[:, 2048:N], t["qTP"][:, 2048:N])
        nc.scalar.dma_start(qc2_b[:, 0:16, :], t["qc2"][:, 0:16, :])
        nc.scalar.dma_start(qc2_b[:, 16:NJC, :], t["qc2"][:, 16:NJC, :])
        nc.gpsimd.dma_start(qc_f[:], t["qcf"])
        nc.gpsimd.dma_start(pk[:], t["pk"])
        nc.gpsimd.dma_start(pkr[:], t["pkr"])
        nc.gpsimd.dma_start(pkb[:], t["pkb"])
        nc.gpsimd.dma_start(qTloc_r[:], t["qTloc"])

        relu_pos = wp.tile([C, NL], f32)
        relu_ch = wp.tile([C, NL], f32)
        sumT = wp.tile([C, NL], f32)
        paF = wp.tile([C, NL + 4], bf16)
        caF = wp.tile([C, NL + 2 * GAP], bf16)
        tmp_ca = wp.tile([C, NL], f32)

        def ccol(x):
            # caF column coordinate with the inter-half gap
            return x + GAP if x >= NL // 2 else x

        # zero the conv-window pads (block tails + inter-half gap)
        nc.vector.memset(paF[:, NL:NL + 4], 0.0)
        nc.vector.memset(caF[:, NL // 2:NL // 2 + GAP], 0.0)
        nc.vector.memset(caF[:, NL + GAP:NL + 2 * GAP], 0.0)

        with tc.tile_pool(name="psAV", bufs=1, space=PSUM) as psAV:
            av0 = psAV.tile([128, 512], f32, tag="avA")  # half A
            # gram quadrants: col-group c accumulates jc = c (mod 4); rhs
            # includes the ones column (N=33) so col 32 accumulates csum --
            # no separate 4.4us DVE reduce over qTP.
            g_ps = psAV.tile([128, C + 1], f32, tag="g")

            def emit_gram(j0, j1):
                for jc in range(j0, j1):
                    cq = jc & 3
                    nc.tensor.matmul(
                        g_ps[32 * cq:32 * cq + C, :],
                        qc_f[:, jc, 0:C], qc_f[:, jc, 0:C + 1],
                        start=(jc < 4), stop=(jc >= NJC - 4),
                        tile_position=(0, 32 * cq), skip_group_check=True,
                    )

            # small1 staged across hooks so no PE matmul waits on a DVE op
            # emitted in the same hook (each stage's MM deps resolved ~2
            # exp-tiles earlier).
            sm1 = {}

            def s1a():
                g_sb = wp.tile([C + 1, C + 1], f32)
                gq1 = wp.tile([C, C + 1], f32)
                gq2 = wp.tile([C, C + 1], f32)
                gq3 = wp.tile([C, C + 1], f32)
                nc.vector.tensor_copy(gq1[:], g_ps[C:2 * C, :])
                nc.vector.tensor_copy(gq2[:], g_ps[2 * C:3 * C, :])
                nc.vector.tensor_copy(gq3[:], g_ps[3 * C:4 * C, :])
                nc.vector.tensor_add(gq1[:], g_ps[0:C, :], gq1[:])
                nc.vector.tensor_add(gq2[:], gq2[:], gq3[:])
                nc.vector.tensor_add(g_sb[0:C, :], gq1[:], gq2[:])
                csum = wp.tile([C, C], f32)
                nc.vector.memset(csum[:], 0.0)
                nc.vector.tensor_copy(csum[:, 0:1], g_sb[0:C, C:C + 1])
                csumT = wp.tile([C, C], f32)
                nc.vector.transpose(csumT[:], csum[:])
                nc.vector.tensor_copy(g_sb[C:C + 1, 0:C], csumT[0:1, :])
                nc.vector.memset(g_sb[C:C + 1, C:C + 1], float(N))
                sm1["g_sb"] = g_sb

            def s1b():
                t1_ps = psAV.tile([C + 1, C], f32, tag="g")
                nc.tensor.matmul(t1_ps[:], sm1["g_sb"][:], wk_f,
                                 start=True, stop=True)
                t1_sb = wp.tile([C + 1, C], f32)
                nc.vector.tensor_copy(t1_sb[:], t1_ps[:])
                sm1["t1_sb"] = t1_sb

            def s1c():
                e2_ps = psAV.tile([C, C], f32, tag="g")
                nc.tensor.matmul(e2_ps[:], wq_f, sm1["t1_sb"][:],
                                 start=True, stop=True)
                mx = wp.tile([C, 1], f32)
                nc.vector.reduce_max(mx[:], e2_ps[:], axis=mybir.AxisListType.X)
                nmx = wp.tile([C, 1], f32)
                nc.vector.tensor_scalar_mul(nmx[:], mx[:], -1.0)
                a_sb = wp.tile([C, C], f32)
                nc.scalar.activation(a_sb[:], e2_ps[:], FT.Exp, bias=nmx[:])
                sm = wp.tile([C, 1], f32)
                nc.vector.reduce_sum(sm[:], a_sb[:], axis=mybir.AxisListType.X)
                rc = wp.tile([C, 1], f32)
                nc.vector.reciprocal(rc[:], sm[:])
                a_n = wp.tile([C, C], f32r)
                nc.vector.tensor_scalar_mul(a_n[:], a_sb[:], rc[:])
                sm1["a_n"] = a_n

            def s1d():
                at_ps = psAV.tile([C, C], f32, tag="g")
                nc.tensor.matmul(at_ps[:], sm1["a_n"][:], id32_r,
                                 start=True, stop=True)
                at_r = wp.tile([C, C], f32r)
                nc.vector.tensor_copy(at_r[:], at_ps[:])
                sm1["at_r"] = at_r

            def s1e():
                wpa_ps = psAV.tile([C + 1, C], f32, tag="g")
                nc.tensor.matmul(wpa_ps[:], wvT_r, sm1["at_r"][:],
                                 start=True, stop=True)
                W_r = wp.tile([C + 1, C], f32r)
                nc.vector.scalar_tensor_tensor(
                    W_r[:], wpa_ps[:], gamma_v, id33_f,
                    op0=ALU.mult, op1=ALU.add,
                )
                sm1["W_r"] = W_r

            def emit_small2():
                """paF = W^T q_aug  (gamma and residual baked into W)."""
                for g in range(4):
                    pa_ps = psAV.tile([C, 512], f32, tag="g")
                    nc.tensor.matmul(
                        pa_ps[:], sm1["W_r"][:],
                        qTloc_r[:, g * 512:(g + 1) * 512],
                        start=True, stop=True,
                    )
                    nc.scalar.copy(paF[0:C, g * 512:(g + 1) * 512], pa_ps[:])

            # ================= big branch: two half-phases =================
            s_roll = [0]   # rolling PE row-group for S matmuls

            def emit_av(av_t, jc, pt_ap):
                for p in range(2):
                    nc.tensor.matmul(
                        av_t[64 * p:64 * p + 64, :],
                        qc2_b[:, jc, 64 * p:64 * p + 64],
                        pt_ap[:, 512 * p:512 * p + 512],
                        start=(jc == 0), stop=(jc == NJC - 1),
                        tile_position=(0, 64 * p), skip_group_check=True,
                    )

            def emit_half(h, avkey, psS, ptp, dve_tiles, hooks, carry_in):
                """S + exp + AV for cols [1024h, 1024h+1024). Whole-tile exp
                alternates ACT/DVE by dve_tiles. AV lags one 3-tile batch;
                the last batch's AVs carry over into the next half's stream
                (no half-boundary drain)."""
                hooks = dict(hooks)
                pend = list(carry_in)
                LAG = 6

                def s_tile(jc):
                    s_ps = psS.tile([128, 1024], f32, tag="s")
                    for s in range(2):
                        rp = s_roll[0] & 3
                        s_roll[0] += 1
                        cg = (2 * h + s) * 512
                        nc.tensor.matmul(
                            s_ps[:, s * 512:(s + 1) * 512],
                            qTP_r[32 * rp:32 * rp + C, jc * 128:(jc + 1) * 128],
                            qTP_r[32 * rp:32 * rp + C, cg:cg + 512],
                            start=True, stop=True,
                            tile_position=(32 * rp, 0), skip_group_check=True,
                        )
                    return s_ps

                def exp_tile(jc, s_ps):
                    pt = ptp.tile([128, 1024], i16, tag="pt")
                    if dve_tiles[jc]:
                        nc.vector.tensor_scalar(
                            pt[:], s_ps[:], A16, B16, op0=ALU.mult, op1=ALU.add,
                        )
                    else:
                        nc.scalar.activation(pt[:].bitcast(bf16), s_ps[:],
                                             FT.Exp)
                    return (jc, avkey, pt[:].bitcast(bf16))

                t0 = 0
                while t0 < NJC:
                    bs = min(3, NJC - t0)
                    sps = [s_tile(t0 + i) for i in range(bs)]
                    cur = [exp_tile(t0 + i, sps[i]) for i in range(bs)]
                    pend += cur
                    ne = max(0, len(pend) - LAG)
                    for jc, ak, pt in pend[:ne]:
                        emit_av(av_box[ak], jc, pt)
                    pend = pend[ne:]
                    for i in range(bs):
                        if t0 + i in hooks:
                            hooks.pop(t0 + i)()
                    t0 += bs
                for k in sorted(hooks):
                    hooks.pop(k)()
                return pend

            def emit_norm(h, av_t, add_eng):
                """caF[0:C, cols h] = av_data/denom + q  (beta pre-folded).
                1/denom via the one-op exponent-flip bit trick: rec =
                bitcast_f32(round(C_MAGIC - float(bits(denom)))), ~±5.5%
                which lands at ~1.4e-3 of output absmax (beta*ca is small)."""
                c0 = h * 1024
                for p in ((1, 0) if h else (0, 1)):
                    rec = wp.tile([C, 512], mybir.dt.int32, tag=f"rec{h}{p}")
                    nc.vector.tensor_scalar(
                        rec[:],
                        av_t[64 * p + C:64 * p + 2 * C, :].bitcast(mybir.dt.int32),
                        -float(0x7EF20790), -1.0, op0=ALU.add, op1=ALU.mult,
                    )
                    nc.vector.tensor_mul(
                        tmp_ca[:, c0 + p * 512:c0 + (p + 1) * 512],
                        av_t[64 * p:64 * p + C, :], rec[:].bitcast(f32),
                    )
                    add_eng.tensor_add(
                        caF[0:C, ccol(c0 + p * 512):ccol(c0 + p * 512) + 512],
                        tmp_ca[:, c0 + p * 512:c0 + (p + 1) * 512],
                        qTloc_f[:, c0 + p * 512:c0 + (p + 1) * 512],
                    )

            def conv1(tag, cu, br, cg, relu_eng):
                """one 512-chunk 4-accum kshift conv into col-group cg of a
                psAV tile, then fused relu+bias; alternate cg across calls so
                consecutive chunks overlap on the PE."""
                cv = psAV.tile([64, 512], f32, tag=tag)
                w4 = wch4_flat if br == "ch" else wpos4_flat
                x2t, base = (caF, ccol(cu * 512)) if br == "ch" \
                    else (paF, cu * 512)
                for k in range(4):
                    nc.tensor.matmul(
                        cv[32 * cg:32 * cg + C, :],
                        w4[:, k * C:(k + 1) * C],
                        x2t[0:C, base + k:base + k + 512],
                        start=(k == 0), stop=(k == 3),
                        tile_position=(0, 32 * cg), skip_group_check=True,
                    )
                relu_out = relu_ch if br == "ch" else relu_pos
                bias = bch_v if br == "ch" else bpos_v
                sl = relu_out[:, cu * 512:(cu + 1) * 512]
                if relu_eng == "act":
                    nc.scalar.activation(sl, cv[32 * cg:32 * cg + C, :],
                                         FT.Relu, bias=bias)
                else:
                    nc.vector.tensor_scalar(
                        sl, cv[32 * cg:32 * cg + C, :], bias, 0.0,
                        op0=ALU.add, op1=ALU.max,
                    )

            def emit_outs(h, add_eng):
                for cu in (2 * h, 2 * h + 1):
                    add_eng.tensor_add(
                        sumT[:, cu * 512:(cu + 1) * 512],
                        relu_ch[:, cu * 512:(cu + 1) * 512],
                        relu_pos[:, cu * 512:(cu + 1) * 512],
                    )
                    nc.sync.dma_start(out_d[:, cu * 512:(cu + 1) * 512],
                                      sumT[:, cu * 512:(cu + 1) * 512])

            av_box = {"av0": av0}
            with (
                tc.tile_pool(name="psS", bufs=3, space=PSUM) as psS,
                tc.tile_pool(name="ptp", bufs=12) as ptp,
            ):
                # gram burst fills the qTP DMA wait (qcf lands first) and
                # warms the HAM before the stream starts.
                emit_gram(0, 8)
                hooksA = {
                    2: lambda: emit_gram(8, 16),
                    4: lambda: emit_gram(16, 24),
                    6: lambda: emit_gram(24, 32),
                    8: s1a, 10: s1b, 12: s1c, 14: s1d, 16: s1e,
                    18: emit_small2,
                    22: lambda: conv1("g", 0, "pos", 0, "act"),
                    26: lambda: conv1("g", 1, "pos", 1, "vec"),
                }
                patA = _dve_pattern(NJC, N_DVE_PER_HALF)
                carry = emit_half(0, "av0", psS, ptp, patA, hooksA, [])

                def normA_mk_av1():
                    # carry AVs for half A were just emitted (batch 0 of
                    # half B): close out half A and open av1 in the slot.
                    emit_norm(0, av0, nc.gpsimd)
                    av1 = psAV.tile([128, 512], f32, tag="avA")
                    av_box["av1"] = av1
                hooksB = {
                    0: normA_mk_av1,
                    5: lambda: conv1("g", 0, "ch", 0, "act"),
                    9: lambda: conv1("g", 1, "ch", 1, "vec"),
                    13: lambda: emit_outs(0, nc.gpsimd),
                    19: lambda: conv1("g", 2, "pos", 0, "act"),
                    23: lambda: conv1("g", 3, "pos", 1, "vec"),
                }
                patB = _dve_pattern(NJC, N_DVE_PER_HALF)
                carry = emit_half(1, "av1", psS, ptp, patB, hooksB, carry)
                for jc, ak, pt in carry:
                    emit_av(av_box[ak], jc, pt)
                # tail: p1 norm first so the cu=3 ch-conv (p1-only window)
                # overlaps the p0 norm chain; cu3 borrows the avA slot.
                emit_norm(1, av_box["av1"], nc.vector)
                conv1("avA", 3, "ch", 1, "act")
                conv1("g", 2, "ch", 0, "act")
                emit_outs(1, nc.vector)


def _build():
    nc = bacc.Bacc("TRN2", target_bir_lowering=False, debug=False)
    t = {}

    def din(name, shape, dt):
        t[name] = nc.dram_tensor(name, shape, dt, kind="ExternalInput").ap()

    din("qTP", [128, N], bf16)
    din("qTloc", [C + 1, NL], f32r)
    din("qc2", [128, NJC, 128], bf16)
    din("qcf", [128, NJC, C + 1], f32)
    din("pk", [128, PKF], f32)
    din("pkr", [C, PRF], f32r)
    din("pkb", [C, PBF], bf16)
    out_d = nc.dram_tensor("out", [C, NL], f32, kind="ExternalOutput").ap()

    with tile.TileContext(nc) as tc:
        _emit(tc, nc, t, out_d)
    nc.compile()
    return nc


_NC = None


def _get_nc():
    global _NC
    if _NC is None:
        _NC = _build()
    return _NC


def _prepare_in_maps(inputs):
    x = np.asarray(inputs["inputs"], np.float32)
    beta = float(np.asarray(inputs["beta"], np.float32)[0])
    gamma = float(np.asarray(inputs["gamma"], np.float32)[0])
    wq_aug = np.concatenate(
        [np.asarray(inputs["wq"], np.float32), np.asarray(inputs["bq"], np.float32)[None, :]], 0
    )
    wk_aug = np.concatenate(
        [np.asarray(inputs["wk"], np.float32), np.asarray(inputs["bk"], np.float32)[None, :]], 0
    )
    wv_aug = np.concatenate(
        [np.asarray(inputs["wv"], np.float32), np.asarray(inputs["bv"], np.float32)[None, :]], 0
    )
    pk = np.zeros((128, PKF), np.float32)
    pk[0:C + 1, PK_WQ:PK_WQ + C] = wq_aug
    pk[0:C + 1, PK_WK:PK_WK + C] = wk_aug
    pk[0:C, PK_SMALL] = np.asarray(inputs["b_ch"], np.float32)
    pk[0:C, PK_SMALL + 1] = np.asarray(inputs["b_pos"], np.float32)
    pk[:, PK_SMALL + 2] = gamma
    pk[0:C, PK_ID33:PK_ID33 + C] = np.eye(C, dtype=np.float32)
    pkr = np.zeros((C, PRF), np.float32)
    pkr[0:C, PR_WVT:PR_WVT + C + 1] = wv_aug.T
    pkr[0:C, PR_ID32:PR_ID32 + C] = np.eye(C, dtype=np.float32)
    pkb = np.zeros((C, PBF), np.float32)
    wch3 = np.asarray(inputs["w_ch"], np.float32).reshape(4, C, C)
    wpos3 = np.asarray(inputs["w_pos"], np.float32).reshape(4, C, C)
    pkb[0:C, PB_WCH4:PB_WCH4 + 4 * C] = wch3.transpose(1, 0, 2).reshape(C, 4 * C)
    pkb[0:C, PB_WPOS4:PB_WPOS4 + 4 * C] = wpos3.transpose(1, 0, 2).reshape(C, 4 * C)
    pkb = pkb.astype(ml_dtypes.bfloat16)

    in_maps = []
    for core in range(NCORES):
        b, s = core // 2, core % 2
        qs = x[b].reshape(N, C)
        # local-half-first column permutation: S_T rhs slices [0, NL) are the
        # core's own rows; softmax sums over all j are order-invariant.
        q = np.concatenate([qs[s * NL:(s + 1) * NL], qs[(1 - s) * NL:(2 - s) * NL]])
        q_aug = np.concatenate([q, np.ones((N, 1), np.float32)], 1)
        qloc_aug = q_aug[:NL]
        qc = np.ascontiguousarray(q_aug.reshape(NJC, 128, C + 1).transpose(1, 0, 2))
        qT_b = np.ascontiguousarray(q.T).astype(ml_dtypes.bfloat16)
        qTP = np.tile(qT_b, (4, 1))
        qc2 = np.ones((128, NJC, 128), np.float32)
        qc2[:, :, 0:C] = beta * qc[:, :, :C]
        qc2[:, :, 2 * C:3 * C] = beta * qc[:, :, :C]
        m = {
            "qTP": qTP,
            "qTloc": np.ascontiguousarray(qloc_aug.T),
            "qc2": qc2.astype(ml_dtypes.bfloat16),
            "qcf": qc,
            "pk": pk,
            "pkr": pkr,
            "pkb": pkb,
        }
        in_maps.append(m)
    return in_maps


def _run(inputs, trace=False):
    nc = _get_nc()
    in_maps = _prepare_in_maps(inputs)
    res = bass_utils.run_bass_kernel_spmd(
        nc, in_maps, core_ids=list(range(NCORES)), trace=trace
    )
    out = np.empty((B, H, W, DO, C), np.float32)
    for core in range(NCORES):
        b, s = core // 2, core % 2
        full = np.ascontiguousarray(res.results[core]["out"].T).reshape(8, W, D, C)
        out[b, s * 8:(s + 1) * 8] = full[:, :, :DO, :]
    return out, res


def kernel(**inputs):
    out, _ = _run(inputs, trace=False)
    return out
